# revision 1
# baseline (speedup 1.0000x reference)
"""DPLSTMCell Trainium2 kernel.

Data-parallel LSTM cell over 8 NeuronCores: batch dim of input/h_prev/c_prev
is sharded, the (small) weights are replicated.

Host-side prep (not part of HW exec time):
  - xh    = concat(input, h_prev) along features, transposed to [K, B] fp16
            so the contraction dim K lands on SBUF partitions.
  - W     = concat(W_ih, W_hh) along features, columns reordered so that each
            "quarter" of the gate dim holds a full (i|f|o|g) set for a
            contiguous slice of output dims, then transposed to [K, 4H] fp16.
  - bias  = (b_ih + b_hh), same column reorder, replicated to [128, 4H] fp32.
Device kernel (per core, B_loc = B/8):
  gates[b,g] = xh @ W^T via PE matmul (fp16 in, fp32 PSUM accum) into PSUM
  quarter tiles [128, H]; bias added on the vector engine; sigmoid/tanh on
  the scalar (ACT) engine; c/h elementwise on the vector engine (DVE); fp32
  in/out for c_prev/h_t/c_t.  Quarter 0 runs k-outer over two 4-wide batch
  groups so matmuls start while W streams in; later quarters are prefetched
  (double-buffered W quarter) and run dense per-batch-tile chains.
"""

import numpy as np

import concourse.bacc as bacc
import concourse.mybir as mybir
import concourse.tile as tile
from concourse.bass_utils import run_bass_kernel_spmd

AF = mybir.ActivationFunctionType
F16 = mybir.dt.float16
F32 = mybir.dt.float32

N_CORES = 8
B_TOTAL = 8192
IN_DIM = 1024
H_DIM = 1024
P = 128


def build_lstm_nc(b_loc=B_TOTAL // N_CORES, in_dim=IN_DIM, h_dim=H_DIM,
                  mm_dtype=F16):
    ktot = in_dim + h_dim
    KT = ktot // P              # contraction tiles
    G = 4 * h_dim               # total gate width
    NQ = 4                      # quarters (gate-interleaved column groups)
    QW = G // NQ                # quarter width (== h_dim)
    DS = h_dim // NQ            # output-dim slice per quarter
    NW = min(512, QW)           # matmul moving free width (PSUM bank limit)
    NCH = QW // NW              # matmul chunks per quarter
    BT = b_loc // P             # batch tiles per core
    GRP = min(4, BT)            # batch tiles in flight for k-outer quarter 0

    nc = bacc.Bacc("TRN2", target_bir_lowering=False)
    xhT = nc.dram_tensor("xhT", [ktot, b_loc], mm_dtype, kind="ExternalInput")
    wT = nc.dram_tensor("wT", [ktot, G], mm_dtype, kind="ExternalInput")
    bias = nc.dram_tensor("bias", [P, G], F32, kind="ExternalInput")
    c_prev = nc.dram_tensor("c_prev", [b_loc, h_dim], F32, kind="ExternalInput")
    h_out = nc.dram_tensor("h_out", [b_loc, h_dim], F32, kind="ExternalOutput")
    c_out = nc.dram_tensor("c_out", [b_loc, h_dim], F32, kind="ExternalOutput")

    with tile.TileContext(nc) as tc:
        with (
            tc.tile_pool(name="const", bufs=1) as const_pool,
            tc.tile_pool(name="xh", bufs=1) as xh_pool,
            tc.tile_pool(name="wt", bufs=2) as wt_pool,
            tc.tile_pool(name="work", bufs=3) as work,
            tc.tile_pool(name="psum", bufs=4, space="PSUM") as psum_pool,
        ):
            xh_sb = xh_pool.tile([P, KT * b_loc], mm_dtype)
            wt_tiles = {}

            def load_wt_quarter(q, interleave_xh=False):
                wt_q = wt_pool.tile([P, KT * QW], mm_dtype, name="wt_q")
                wt_tiles[q] = wt_q
                hb = min(GRP * P, b_loc)
                for k in range(KT):
                    if interleave_xh and k == 0:
                        # split the very first transfers so the first batch
                        # group's k0 matmuls unblock before the bulk traffic
                        # piles up on the DMA engines (completion semaphores
                        # fire only when a transfer's last packet drains);
                        # xh's second batch half (group 1, needed ~25us
                        # later) is deferred past k3 to speed k1-k3 arrival
                        nc.sync.dma_start(wt_q[:, 0:NW],
                                          wT[0:P, q * QW:q * QW + NW])
                        nc.sync.dma_start(xh_sb[:, 0:hb], xhT[0:P, 0:hb])
                        if NW < QW:
                            nc.sync.dma_start(
                                wt_q[:, NW:QW],
                                wT[0:P, q * QW + NW:(q + 1) * QW])
                        continue
                    nc.sync.dma_start(
                        wt_q[:, k * QW:(k + 1) * QW],
                        wT[k * P:(k + 1) * P, q * QW:(q + 1) * QW])
                    if interleave_xh:
                        nc.sync.dma_start(
                            xh_sb[:, k * b_loc:(k + 1) * b_loc],
                            xhT[k * P:(k + 1) * P, :])
                        if k == min(3, KT - 1) and hb < b_loc:
                            nc.sync.dma_start(xh_sb[:, hb:b_loc],
                                              xhT[0:P, hb:b_loc])

            # quarter 0 W and the transposed activations, interleaved k-wise
            # so the first accumulation chains can start immediately.
            load_wt_quarter(0, interleave_xh=True)

            # bias loaded per quarter so the 2MB transfer doesn't sit in the
            # DMA queue ahead of quarter 0's c_prev loads
            bias_sb = const_pool.tile([P, G], F32)
            nc.sync.dma_start(bias_sb[:, 0:QW], bias[:, 0:QW])

            # PE warmup: ~3.4us of dummy matmuls on zeroed SBUF while the
            # first W/xh tiles stream in, so HAM is at K=8/8 (2.4 GHz) when
            # real matmuls start.
            scratch = work.tile([P, NW], mm_dtype, name="scratch", bufs=1)
            nc.vector.memset(scratch[:], 0.0)
            zb = const_pool.tile([P, 1], F32)
            nc.vector.memset(zb[:], 0.0)
            ps_w = psum_pool.tile([P, QW], F32, name="ps")
            for i in range(8):
                nc.tensor.matmul(
                    ps_w[:, (i % NCH) * NW:(i % NCH + 1) * NW],
                    scratch[:, 0:P], scratch[:],
                    start=True, stop=True)

            def mm_pair(ps, q, k, b):
                xsl = xh_sb[:, k * b_loc + b * P:k * b_loc + (b + 1) * P]
                wt_q = wt_tiles[q]
                for c in range(NCH):
                    nc.tensor.matmul(
                        ps[:, c * NW:(c + 1) * NW],
                        xsl,
                        wt_q[:, k * QW + c * NW:k * QW + (c + 1) * NW],
                        start=(k == 0), stop=(k == KT - 1))

            def bias_add(ps, q):
                # gates = psum + bias on the DVE. This is the ONLY psum
                # reader, so the PSUM slot frees right after it; emitted for
                # a whole batch group before the rest of the epilogues so the
                # in-order DVE doesn't hold PSUM hostage behind ACT waits.
                gates = work.tile([P, QW], F32, name="gates", bufs=8)
                nc.vector.tensor_add(
                    gates[:], ps[:], bias_sb[:, q * QW:(q + 1) * QW])
                return gates

            def epilogue_tail(gates, q, b):
                # everything past the gate activations; shared with last_tile
                cp = work.tile([P, DS], F32, name="cp")
                nc.sync.dma_start(
                    cp[:], c_prev[b * P:(b + 1) * P, q * DS:(q + 1) * DS])

                ig = work.tile([P, DS], F32, name="ig")
                nc.vector.tensor_mul(ig[:], gates[:, 0:DS],
                                     gates[:, 3 * DS:4 * DS])
                cnew = work.tile([P, DS], F32, name="cnew")
                nc.vector.tensor_mul(cnew[:], gates[:, DS:2 * DS], cp[:])
                nc.vector.tensor_add(cnew[:], cnew[:], ig[:])
                tct = work.tile([P, DS], F32, name="tct")
                nc.scalar.activation(tct[:], cnew[:], AF.Tanh, bias=zb[:])
                hnew = work.tile([P, DS], F32, name="hnew")
                nc.vector.tensor_mul(hnew[:], gates[:, 2 * DS:3 * DS], tct[:])

                nc.sync.dma_start(
                    c_out[b * P:(b + 1) * P, q * DS:(q + 1) * DS], cnew[:])
                nc.sync.dma_start(
                    h_out[b * P:(b + 1) * P, q * DS:(q + 1) * DS], hnew[:])

            def epilogue(gates, q, b):
                # quarter layout: [ i | f | o | g ], each DS wide
                nc.scalar.activation(gates[:, 0:3 * DS], gates[:, 0:3 * DS],
                                     AF.Sigmoid, bias=zb[:])
                nc.scalar.activation(gates[:, 3 * DS:4 * DS],
                                     gates[:, 3 * DS:4 * DS], AF.Tanh,
                                     bias=zb[:])
                epilogue_tail(gates, q, b)

            def last_tile(q, b):
                # Final tile: skew the two 512-wide chunks by LAG k-steps
                # (keeping bank alternation) so the [i|f] half's bias-add and
                # sigmoid overlap the [o|g] half's remaining matmuls.
                LAG = 4
                ps = psum_pool.tile([P, QW], F32, name="ps")
                wt_q = wt_tiles[q]
                for j in range(KT + LAG):
                    for c, k in ((0, j), (1, j - LAG)):
                        if 0 <= k < KT:
                            xsl = xh_sb[:, k * b_loc + b * P:
                                        k * b_loc + (b + 1) * P]
                            nc.tensor.matmul(
                                ps[:, c * NW:(c + 1) * NW],
                                xsl,
                                wt_q[:, k * QW + c * NW:
                                     k * QW + (c + 1) * NW],
                                start=(k == 0), stop=(k == KT - 1))
                gates = work.tile([P, QW], F32, name="gates", bufs=8)
                nc.vector.tensor_add(
                    gates[:, 0:NW], ps[:, 0:NW],
                    bias_sb[:, q * QW:q * QW + NW])
                nc.scalar.activation(gates[:, 0:2 * DS], gates[:, 0:2 * DS],
                                     AF.Sigmoid, bias=zb[:])
                nc.vector.tensor_add(
                    gates[:, NW:2 * NW], ps[:, NW:2 * NW],
                    bias_sb[:, q * QW + NW:q * QW + 2 * NW])
                nc.scalar.activation(gates[:, 2 * DS:3 * DS],
                                     gates[:, 2 * DS:3 * DS],
                                     AF.Sigmoid, bias=zb[:])
                nc.scalar.activation(gates[:, 3 * DS:4 * DS],
                                     gates[:, 3 * DS:4 * DS], AF.Tanh,
                                     bias=zb[:])
                epilogue_tail(gates, q, b)

            # ---- quarter 0: k-outer over GRP-wide batch groups ----
            for g0 in range(0, BT, GRP):
                pss = [psum_pool.tile([P, QW], F32, name="ps")
                       for _ in range(min(GRP, BT - g0))]
                for k in range(KT):
                    for bi, ps in enumerate(pss):
                        mm_pair(ps, 0, k, g0 + bi)
                gts = [bias_add(ps, 0) for ps in pss]
                for bi, gates in enumerate(gts):
                    epilogue(gates, 0, g0 + bi)

            # ---- quarters 1..: prefetched, dense per-b chains ----
            for q in range(1, NQ):
                load_wt_quarter(q)
                nc.sync.dma_start(bias_sb[:, q * QW:(q + 1) * QW],
                                  bias[:, q * QW:(q + 1) * QW])
                for b in range(BT):
                    if q == NQ - 1 and b == BT - 1 and NCH == 2:
                        last_tile(q, b)
                        continue
                    ps = psum_pool.tile([P, QW], F32, name="ps")
                    for k in range(KT):
                        mm_pair(ps, q, k, b)
                    epilogue(bias_add(ps, q), q, b)

    nc.compile()
    return nc


def prep_inputs(input, h_prev, c_prev, W_ih, b_ih, W_hh, b_hh,
                n_cores=N_CORES, np_mm_dtype=np.float16):
    """Host-side shard + layout prep. Returns list of per-core input maps."""
    input = np.asarray(input, np.float32)
    h_prev = np.asarray(h_prev, np.float32)
    c_prev = np.asarray(c_prev, np.float32)
    W_ih = np.asarray(W_ih, np.float32)
    W_hh = np.asarray(W_hh, np.float32)
    b_ih = np.asarray(b_ih, np.float32)
    b_hh = np.asarray(b_hh, np.float32)

    b_total, _ = input.shape
    h_dim = h_prev.shape[1]
    b_loc = b_total // n_cores
    G = 4 * h_dim
    NQ = 4
    DS = h_dim // NQ

    # column reorder: per quarter q the layout is [i | f | o | g] for output
    # dims [q*DS, (q+1)*DS)
    arr = np.arange(G).reshape(4, NQ, DS)       # [gate, q, r]
    idx = arr[[0, 1, 3, 2]].transpose(1, 0, 2).reshape(-1)

    W_cat = np.concatenate([W_ih, W_hh], axis=1)            # [G, ktot]
    wT = np.ascontiguousarray(W_cat[idx, :].T, dtype=np_mm_dtype)
    bias_row = (b_ih + b_hh)[idx].astype(np.float32)
    bias = np.ascontiguousarray(np.broadcast_to(bias_row, (128, G)))

    xh = np.concatenate([input, h_prev], axis=1)            # [B, ktot]
    xhT = xh.T                                              # [ktot, B] (view)

    in_maps = []
    for c in range(n_cores):
        in_maps.append({
            "xhT": np.ascontiguousarray(
                xhT[:, c * b_loc:(c + 1) * b_loc], dtype=np_mm_dtype),
            "wT": wT,
            "bias": bias,
            "c_prev": np.ascontiguousarray(c_prev[c * b_loc:(c + 1) * b_loc]),
        })
    return in_maps


def run_lstm(inputs, trace=False, **spmd_kwargs):
    """Builds + runs the kernel on all 8 cores. Returns (h_t, c_t), results."""
    in_maps = prep_inputs(**inputs)
    nc = build_lstm_nc()
    res = run_bass_kernel_spmd(nc, in_maps, core_ids=list(range(N_CORES)),
                               trace=trace, **spmd_kwargs)
    h_t = np.concatenate([r["h_out"] for r in res.results], axis=0)
    c_t = np.concatenate([r["c_out"] for r in res.results], axis=0)
    return (h_t, c_t), res


def kernel(input, h_prev, c_prev, W_ih, b_ih, W_hh, b_hh):
    (h_t, c_t), _ = run_lstm(dict(
        input=input, h_prev=h_prev, c_prev=c_prev,
        W_ih=W_ih, b_ih=b_ih, W_hh=W_hh, b_hh=b_hh))
    return (h_t, c_t)



# revision 2
# speedup vs baseline: 1.1036x; 1.1036x over previous
"""DPLSTMCell Trainium2 kernel — mixed fp8-DoubleRow / fp16.

Data-parallel LSTM cell over 8 NeuronCores: batch dim of input/h_prev/c_prev
is sharded, the (small) weights are replicated.

Numerics: gates = xh @ W^T + bias with K = 2048.  The i/f/o gate columns run
as fp8 (e4m3) DoubleRow matmuls — 2 MACs/cell/cycle on the PE array — while
the tanh g-gate columns, which dominate the quantization error of the final
h_t/c_t, stay in fp16 at normal rate.  Host-side study with exact RNE
rounding: rel_l2 ≈ 1.6e-2 (tolerance 2e-2); all-fp8 would be 2.6e-2, all-fp16
2e-4 but ~1.5x slower.  x and W are pre-scaled by 32 before the fp8 cast
(avoids the subnormal region); the combined 1024x comes back out via the
activation's scale operand, and the fp8-side bias is pre-multiplied by 1024.

Layout: the 1024 output dims split into 2 halves of DS=512.  Per half the
gate columns are grouped [i|f|o] (fp8, NF8=1536 cols) + [g] (fp16, 512).
Per (half, b-tile) unit a [128, 2048] PSUM tile (4 banks, 2 units in
flight) accumulates: fp8 via 8 DoubleRow k-pair steps x 3 N=512 MMs, fp16
via 16 k steps x 1 MM.  Epilogue: DVE bias-adds (the only PSUM readers),
ACT sigmoid (scale=1/1024) / tanh, DVE elementwise c/h math, DMA out.

The first two units run their fp8 phase k-outer (12 MMs per k-pair across
both PSUM buffers) so the PE consumes W/x tiles at the rate DMA delivers
them; warmup dummy MMs ramp the HAM clock gate to 8/8 meanwhile.
"""

import ml_dtypes
import numpy as np

import concourse.bacc as bacc
import concourse.mybir as mybir
import concourse.tile as tile
from concourse.bass_utils import run_bass_kernel_spmd

AF = mybir.ActivationFunctionType
F8 = mybir.dt.float8e4
F16 = mybir.dt.float16
F32 = mybir.dt.float32
DR = mybir.MatmulPerfMode.DoubleRow

N_CORES = 8
B_TOTAL = 8192
IN_DIM = 1024
H_DIM = 1024
P = 128

B_LOC = B_TOTAL // N_CORES   # 1024
BT = B_LOC // P              # 8 batch tiles
KTOT = IN_DIM + H_DIM        # 2048
KT = KTOT // P               # 16 k tiles
KP = KT // 2                 # 8 DoubleRow k pairs
DS = 512                     # output-dim slice per half
NH = H_DIM // DS             # 2 halves

SX = 32.0                    # fp8 scale on x
SW = 32.0                    # fp8 scale on W
SCALE = SX * SW              # folded back via activation scale

# PROMOTE_O=False: fp8 [i|f|o], fp16 [g].  True: fp8 [i|f], fp16 [g|o]
# (accuracy knob: 1.62e-2 -> 1.21e-2 at ~15% more PE time).
PROMOTE_O = False


def _cfg(promote_o=PROMOTE_O):
    if promote_o:
        nf8 = 2 * DS           # [i|f]
        off = dict(i=0, f=DS, g=2 * DS, o=3 * DS)
    else:
        nf8 = 3 * DS           # [i|f|o]
        off = dict(i=0, f=DS, o=2 * DS, g=3 * DS)
    return nf8, 4 * DS - nf8, off


def build_lstm_nc(promote_o=PROMOTE_O):
    NF8, NF16, OFF = _cfg(promote_o)
    C8 = NF8 // DS             # fp8 N=512 chunks per unit
    C16 = NF16 // DS

    nc = bacc.Bacc("TRN2", target_bir_lowering=False)
    x8_d = nc.dram_tensor("x8", [KTOT, B_LOC], F8, kind="ExternalInput")
    x16_d = nc.dram_tensor("x16", [KTOT, B_LOC], F16, kind="ExternalInput")
    w8_d = nc.dram_tensor("w8", [KTOT, NH * NF8], F8, kind="ExternalInput")
    w16_d = nc.dram_tensor("w16", [KTOT, NH * NF16], F16, kind="ExternalInput")
    b8_d = nc.dram_tensor("bias8", [P, NH * NF8], F32, kind="ExternalInput")
    b16_d = nc.dram_tensor("bias16", [P, NH * NF16], F32, kind="ExternalInput")
    cp_d = nc.dram_tensor("c_prev", [B_LOC, H_DIM], F32, kind="ExternalInput")
    h_d = nc.dram_tensor("h_out", [B_LOC, H_DIM], F32, kind="ExternalOutput")
    c_d = nc.dram_tensor("c_out", [B_LOC, H_DIM], F32, kind="ExternalOutput")

    with tile.TileContext(nc) as tc:
        with (
            tc.tile_pool(name="const", bufs=1) as const_pool,
            tc.tile_pool(name="xw", bufs=1) as xw,
            tc.tile_pool(name="work", bufs=3) as work,
            tc.tile_pool(name="psum", bufs=2, space="PSUM") as psum_pool,
        ):
            x8_sb = xw.tile([P, KT, B_LOC], F8, name="x8")
            x16_sb = xw.tile([P, KT, B_LOC], F16, name="x16")
            w8_sb = [xw.tile([P, KT, NF8], F8, name=f"w8_{h}") for h in range(NH)]
            w16_sb = [xw.tile([P, KT, NF16], F16, name=f"w16_{h}")
                      for h in range(NH)]
            b8_sb = const_pool.tile([P, NH * NF8], F32)
            b16_sb = const_pool.tile([P, NH * NF16], F32)

            HB = BT // 2 * P   # first 4 b-tiles' columns of x

            # --- DMA, priority-ordered.  Phase A: everything unit (h0,b0/b1)
            # consumes, k-interleaved so the k-outer warm-up loop can chase
            # the transfers.
            for k in range(KT):
                nc.sync.dma_start(w8_sb[0][:, k, :],
                                  w8_d[k * P:(k + 1) * P, 0:NF8])
                nc.sync.dma_start(x8_sb[:, k, 0:HB],
                                  x8_d[k * P:(k + 1) * P, 0:HB])
            for k in range(KT):
                nc.sync.dma_start(w16_sb[0][:, k, :],
                                  w16_d[k * P:(k + 1) * P, 0:NF16])
                nc.sync.dma_start(x16_sb[:, k, 0:HB],
                                  x16_d[k * P:(k + 1) * P, 0:HB])
            nc.sync.dma_start(b8_sb[:, 0:NF8], b8_d[:, 0:NF8])
            nc.sync.dma_start(b16_sb[:, 0:NF16], b16_d[:, 0:NF16])
            # Phase B: second batch half + half-1 weights + remaining biases.
            for k in range(KT):
                nc.sync.dma_start(x8_sb[:, k, HB:B_LOC],
                                  x8_d[k * P:(k + 1) * P, HB:B_LOC])
                nc.sync.dma_start(x16_sb[:, k, HB:B_LOC],
                                  x16_d[k * P:(k + 1) * P, HB:B_LOC])
            for k in range(KT):
                nc.sync.dma_start(w8_sb[1][:, k, :],
                                  w8_d[k * P:(k + 1) * P, NF8:2 * NF8])
                nc.sync.dma_start(w16_sb[1][:, k, :],
                                  w16_d[k * P:(k + 1) * P, NF16:2 * NF16])
            nc.sync.dma_start(b8_sb[:, NF8:], b8_d[:, NF8:])
            nc.sync.dma_start(b16_sb[:, NF16:], b16_d[:, NF16:])

            # --- PE warmup: ~3.4us of dummy MMs while the first W/x tiles
            # stream in, so HAM is at K=8/8 when real matmuls start.
            scratch = const_pool.tile([P, 5 * P], F16, name="scratch")
            nc.vector.memset(scratch[:], 0.0)

            def mm8(ps, h, kp, b, c):
                nc.tensor.matmul(
                    ps[:, c * DS:(c + 1) * DS],
                    x8_sb[:, 2 * kp:2 * kp + 2, b * P:(b + 1) * P],
                    w8_sb[h][:, 2 * kp:2 * kp + 2, c * DS:(c + 1) * DS],
                    start=(kp == 0), stop=(kp == KP - 1), perf_mode=DR)

            def mm16(ps, h, k, b, c):
                nc.tensor.matmul(
                    ps[:, NF8 + c * DS:NF8 + (c + 1) * DS],
                    x16_sb[:, k, b * P:(b + 1) * P],
                    w16_sb[h][:, k, c * DS:(c + 1) * DS],
                    start=(k == 0), stop=(k == KT - 1))

            def load_cp(h, b):
                cp = work.tile([P, DS], F32, name="cp", bufs=4)
                nc.sync.dma_start(
                    cp[:], cp_d[b * P:(b + 1) * P, h * DS:(h + 1) * DS])
                return cp

            def epilogue(ps, h, b, cp):
                gates = work.tile([P, 4 * DS], F32, name="gates", bufs=3)
                # DVE bias-adds are the only PSUM readers -> free the banks.
                nc.vector.tensor_add(gates[:, 0:NF8], ps[:, 0:NF8],
                                     b8_sb[:, h * NF8:(h + 1) * NF8])
                nc.vector.tensor_add(gates[:, NF8:], ps[:, NF8:],
                                     b16_sb[:, h * NF16:(h + 1) * NF16])
                # fp8 gates carry the 1024x quant scale; sigmoid applies 1/1024.
                nc.scalar.activation(gates[:, 0:NF8], gates[:, 0:NF8],
                                     AF.Sigmoid, scale=1.0 / SCALE)
                if promote_o:
                    nc.scalar.activation(gates[:, OFF["g"]:OFF["g"] + DS],
                                         gates[:, OFF["g"]:OFF["g"] + DS],
                                         AF.Tanh)
                    nc.scalar.activation(gates[:, OFF["o"]:OFF["o"] + DS],
                                         gates[:, OFF["o"]:OFF["o"] + DS],
                                         AF.Sigmoid)
                else:
                    nc.scalar.activation(gates[:, OFF["g"]:OFF["g"] + DS],
                                         gates[:, OFF["g"]:OFF["g"] + DS],
                                         AF.Tanh)
                ig = work.tile([P, DS], F32, name="ig")
                nc.vector.tensor_mul(ig[:], gates[:, OFF["i"]:OFF["i"] + DS],
                                     gates[:, OFF["g"]:OFF["g"] + DS])
                cnew = work.tile([P, DS], F32, name="cnew")
                nc.vector.tensor_mul(cnew[:], gates[:, OFF["f"]:OFF["f"] + DS],
                                     cp[:])
                nc.vector.tensor_add(cnew[:], cnew[:], ig[:])
                tct = work.tile([P, DS], F32, name="tct")
                nc.scalar.activation(tct[:], cnew[:], AF.Tanh)
                hnew = work.tile([P, DS], F32, name="hnew")
                nc.vector.tensor_mul(hnew[:], gates[:, OFF["o"]:OFF["o"] + DS],
                                     tct[:])
                nc.sync.dma_start(
                    c_d[b * P:(b + 1) * P, h * DS:(h + 1) * DS], cnew[:])
                nc.sync.dma_start(
                    h_d[b * P:(b + 1) * P, h * DS:(h + 1) * DS], hnew[:])

            # --- units (h0, b0) and (h0, b1): warmup dummies + k-outer fp8
            # so the PE start chases the W8/x8 transfers.
            ps01 = [psum_pool.tile([P, 4 * DS], F32, name="ps")
                    for _ in range(2)]
            for i in range(8):
                nc.tensor.matmul(ps01[0][:, (i % 2) * DS:(i % 2 + 1) * DS],
                                 scratch[:, 0:P], scratch[:, P:],
                                 start=True, stop=True)
            cp01 = [load_cp(0, 0), load_cp(0, 1)]
            for kp in range(KP):
                for u in range(2):
                    for c in range(C8):
                        mm8(ps01[u], 0, kp, u, c)
            for k in range(KT):
                for u in range(2):
                    for c in range(C16):
                        mm16(ps01[u], 0, k, u, c)
            epilogue(ps01[0], 0, 0, cp01[0])
            epilogue(ps01[1], 0, 1, cp01[1])

            # --- remaining units: dense per-unit chains.
            for h in range(NH):
                for b in range(BT):
                    if h == 0 and b < 2:
                        continue
                    cp = load_cp(h, b)
                    ps = psum_pool.tile([P, 4 * DS], F32, name="ps")
                    for kp in range(KP):
                        for c in range(C8):
                            mm8(ps, h, kp, b, c)
                    for k in range(KT):
                        for c in range(C16):
                            mm16(ps, h, k, b, c)
                    epilogue(ps, h, b, cp)

    nc.compile()
    return nc


def _e4m3(v):
    return np.clip(v, -240.0, 240.0).astype(ml_dtypes.float8_e4m3fn)


def prep_inputs(input, h_prev, c_prev, W_ih, b_ih, W_hh, b_hh,
                n_cores=N_CORES, promote_o=PROMOTE_O):
    """Host-side shard + layout + quantization prep (not in HW exec time)."""
    NF8, NF16, _ = _cfg(promote_o)
    input = np.asarray(input, np.float32)
    h_prev = np.asarray(h_prev, np.float32)
    c_prev = np.asarray(c_prev, np.float32)
    W_cat = np.concatenate([np.asarray(W_ih, np.float32),
                            np.asarray(W_hh, np.float32)], axis=1)  # [G, K]
    bias = (np.asarray(b_ih, np.float32) + np.asarray(b_hh, np.float32))

    H = H_DIM
    blocks8 = [0, 1] if promote_o else [0, 1, 3]   # gate row-blocks i,f,(o)
    blocks16 = [2, 3] if promote_o else [2]        # g,(o)
    idx8, idx16 = [], []
    for hh in range(NH):
        for gb in blocks8:
            idx8 += list(range(gb * H + hh * DS, gb * H + (hh + 1) * DS))
        for gb in blocks16:
            idx16 += list(range(gb * H + hh * DS, gb * H + (hh + 1) * DS))

    w8 = _e4m3(np.ascontiguousarray(W_cat[idx8, :].T) * SW)       # [K, NH*NF8]
    w16 = np.ascontiguousarray(W_cat[idx16, :].T).astype(np.float16)
    bias8 = np.ascontiguousarray(np.broadcast_to(
        bias[idx8] * SCALE, (P, NH * NF8)).astype(np.float32))
    bias16 = np.ascontiguousarray(np.broadcast_to(
        bias[idx16], (P, NH * NF16)).astype(np.float32))

    xh = np.concatenate([input, h_prev], axis=1)    # [B, K]
    xhT = xh.T                                      # [K, B] view

    b_loc = input.shape[0] // n_cores
    in_maps = []
    for c in range(n_cores):
        sl = xhT[:, c * b_loc:(c + 1) * b_loc]
        in_maps.append({
            "x8": _e4m3(np.ascontiguousarray(sl) * SX),
            "x16": np.ascontiguousarray(sl, dtype=np.float16),
            "w8": w8,
            "w16": w16,
            "bias8": bias8,
            "bias16": bias16,
            "c_prev": np.ascontiguousarray(c_prev[c * b_loc:(c + 1) * b_loc]),
        })
    return in_maps


def run_lstm(inputs, trace=False, **spmd_kwargs):
    """Builds + runs the kernel on all 8 cores. Returns (h_t, c_t), results."""
    in_maps = prep_inputs(**inputs)
    nc = build_lstm_nc()
    res = run_bass_kernel_spmd(nc, in_maps, core_ids=list(range(N_CORES)),
                               trace=trace, **spmd_kwargs)
    h_t = np.concatenate([r["h_out"] for r in res.results], axis=0)
    c_t = np.concatenate([r["c_out"] for r in res.results], axis=0)
    return (h_t, c_t), res


def kernel(input, h_prev, c_prev, W_ih, b_ih, W_hh, b_hh):
    (h_t, c_t), _ = run_lstm(dict(
        input=input, h_prev=h_prev, c_prev=c_prev,
        W_ih=W_ih, b_ih=b_ih, W_hh=W_hh, b_hh=b_hh))
    return (h_t, c_t)


# revision 3
# speedup vs baseline: 1.4492x; 1.3131x over previous
"""DPLSTMCell Trainium2 kernel — mixed fp8-DoubleRow / fp16, decoupled pipes.

Data-parallel LSTM cell over 8 NeuronCores: batch dim of input/h_prev/c_prev
is sharded, the (small) weights are replicated.

Numerics: gates = xh @ W^T + bias with K = 2048.  The i/f/o gate columns run
as fp8 (e4m3) DoubleRow matmuls — 2 MACs/cell/cycle — while the tanh g-gate
columns, which dominate the output error, stay fp16 at normal rate.  x and W
are pre-scaled by 32 before the e4m3 cast (dodges subnormals); the combined
1024x comes back out via the sigmoid's scale operand; the fp8-side bias is
pre-multiplied by 1024.  Everything downstream of PSUM is fp16 (verified
host-side: full-chain rel_l2 = 1.62e-2 vs the 2e-2 gate; fp16 stages add
<1e-4).  c_prev/h_out/c_out/biases travel as fp16 to halve DMA bytes.

Per core (B_loc=1024): 16 units = (half h of 512 output dims) x (8 b-tiles).
Two decoupled PE pipelines per unit:
  fp8 pipe: 8 DoubleRow k-pair steps x 3 N=512 MMs into a 3-bank PSUM tile
      -> DVE +bias (frees PSUM) -> ACT sigmoid -> ifo[P,1536] fp16 in SBUF.
  g pipe (staggered GLAG units later, so its fp16 W/x transfers never gate
      the fp8 stream): 16 k steps x 1 MM into a 1-bank PSUM tile -> DVE
      +bias -> ACT tanh -> g[P,512] fp16.
  join: c/h elementwise on DVE in fp16 (2x rate), fp16 DMA out.
PSUM: 2x3 banks (fp8) + 2x1 (g) = 8.  The fp8 chains of units 0/1 run
k-outer, chasing the W8/x8 transfers; warmup dummy MMs ramp the HAM clock
gate to 8/8 meanwhile.  DRAM operands are partition-major so every DMA row
is 2-4KB contiguous.
"""

import ml_dtypes
import numpy as np

import concourse.bacc as bacc
import concourse.mybir as mybir
import concourse.tile as tile
from concourse.bass_utils import run_bass_kernel_spmd

AF = mybir.ActivationFunctionType
F8 = mybir.dt.float8e4
F16 = mybir.dt.float16
F32 = mybir.dt.float32
DR = mybir.MatmulPerfMode.DoubleRow

N_CORES = 8
B_TOTAL = 8192
IN_DIM = 1024
H_DIM = 1024
P = 128

B_LOC = B_TOTAL // N_CORES   # 1024
BT = B_LOC // P              # 8 batch tiles
KTOT = IN_DIM + H_DIM        # 2048
KT = KTOT // P               # 16 k tiles
KP = KT // 2                 # 8 DoubleRow k pairs
DS = 512                     # output-dim slice per half
NH = H_DIM // DS             # 2 halves
NU = NH * BT                 # 16 units

SX = 32.0
SW = 32.0
SCALE = SX * SW

GLAG = 4                     # g pipe trails the fp8 pipe by this many units

# PROMOTE_O=False: fp8 [i|f|o], fp16 [g].  True: fp8 [i|f], fp16 [g|o].
PROMOTE_O = False


def _cfg(promote_o=PROMOTE_O):
    nf8 = 2 * DS if promote_o else 3 * DS
    return nf8, 4 * DS - nf8


def build_lstm_nc(promote_o=PROMOTE_O):
    NF8, NF16 = _cfg(promote_o)
    C8 = NF8 // DS
    C16 = NF16 // DS

    nc = bacc.Bacc("TRN2", target_bir_lowering=False)
    # partition-major layouts: row p holds that partition's data for all k.
    x8_d = nc.dram_tensor("x8", [P, KT * B_LOC], F8, kind="ExternalInput")
    x16_d = nc.dram_tensor("x16", [P, KT * B_LOC], F16, kind="ExternalInput")
    w8_d = nc.dram_tensor("w8", [P, NH * KT * NF8], F8, kind="ExternalInput")
    w16_d = nc.dram_tensor("w16", [P, NH * KT * NF16], F16,
                           kind="ExternalInput")
    b8_d = nc.dram_tensor("bias8", [P, NH * NF8], F16, kind="ExternalInput")
    b16_d = nc.dram_tensor("bias16", [P, NH * NF16], F16, kind="ExternalInput")
    cp_d = nc.dram_tensor("c_prev", [B_LOC, H_DIM], F16, kind="ExternalInput")
    h_d = nc.dram_tensor("h_out", [B_LOC, H_DIM], F16, kind="ExternalOutput")
    c_d = nc.dram_tensor("c_out", [B_LOC, H_DIM], F16, kind="ExternalOutput")

    units = [(h, b) for h in range(NH) for b in range(BT)]

    with tile.TileContext(nc) as tc:
        with (
            tc.tile_pool(name="const", bufs=1) as const_pool,
            tc.tile_pool(name="xw", bufs=1) as xw,
            tc.tile_pool(name="work", bufs=3) as work,
            tc.tile_pool(name="ps8", bufs=2, space="PSUM") as ps8_pool,
            tc.tile_pool(name="psg", bufs=2, space="PSUM") as psg_pool,
        ):
            x8_sb = xw.tile([P, KT, B_LOC], F8, name="x8")
            x16_sb = xw.tile([P, KT, B_LOC], F16, name="x16")
            w8_sb = [xw.tile([P, KT, NF8], F8, name=f"w8_{h}")
                     for h in range(NH)]
            w16_sb = [xw.tile([P, KT, NF16], F16, name=f"w16_{h}")
                      for h in range(NH)]
            b8_sb = const_pool.tile([P, NH * NF8], F16)
            b16_sb = const_pool.tile([P, NH * NF16], F16)

            def dma_w8(h, kp):
                base = h * KT * NF8 + 2 * kp * NF8
                nc.sync.dma_start(w8_sb[h][:, 2 * kp:2 * kp + 2, :],
                                  w8_d[:, base:base + 2 * NF8])

            def dma_w16(h, kp):
                base = h * KT * NF16 + 2 * kp * NF16
                nc.sync.dma_start(w16_sb[h][:, 2 * kp:2 * kp + 2, :],
                                  w16_d[:, base:base + 2 * NF16])

            def dma_x(sb, d, kp):
                nc.sync.dma_start(sb[:, 2 * kp:2 * kp + 2, :],
                                  d[:, 2 * kp * B_LOC:(2 * kp + 2) * B_LOC])

            # phase A: fp8 operands for half 0, k-interleaved; biases late in
            # the stream (first DVE bias-add is ~17us in).
            for kp in range(KP):
                dma_w8(0, kp)
                dma_x(x8_sb, x8_d, kp)
                if kp == 6:
                    nc.sync.dma_start(b8_sb[:, 0:NF8], b8_d[:, 0:NF8])
                    nc.sync.dma_start(b16_sb[:, 0:NF16], b16_d[:, 0:NF16])

            scratch = const_pool.tile([P, 5 * P], F16, name="scratch")
            nc.vector.memset(scratch[:], 0.0)

            def mm8(ps, h, kp, b):
                for c in range(C8):
                    nc.tensor.matmul(
                        ps[:, c * DS:(c + 1) * DS],
                        x8_sb[:, 2 * kp:2 * kp + 2, b * P:(b + 1) * P],
                        w8_sb[h][:, 2 * kp:2 * kp + 2, c * DS:(c + 1) * DS],
                        start=(kp == 0), stop=(kp == KP - 1), perf_mode=DR)

            def fp8_chain(u):
                h, b = units[u]
                ps = ps8_pool.tile([P, NF8], F32, name="ps8")
                for kp in range(KP):
                    mm8(ps, h, kp, b)
                return ps

            def fp8_epi(u, ps):
                h, b = units[u]
                ifo = work.tile([P, NF8], F16, name="ifo", bufs=GLAG + 3)
                nc.vector.tensor_add(ifo[:], ps[:],
                                     b8_sb[:, h * NF8:(h + 1) * NF8])
                nc.scalar.activation(ifo[:], ifo[:], AF.Sigmoid,
                                     scale=1.0 / SCALE)
                return ifo

            def g_chain_and_join(u, ifo):
                h, b = units[u]
                cp = work.tile([P, DS], F16, name="cp")
                nc.sync.dma_start(
                    cp[:], cp_d[b * P:(b + 1) * P, h * DS:(h + 1) * DS])
                ps = psg_pool.tile([P, NF16], F32, name="psg")
                for k in range(KT):
                    for c in range(C16):
                        nc.tensor.matmul(
                            ps[:, c * DS:(c + 1) * DS],
                            x16_sb[:, k, b * P:(b + 1) * P],
                            w16_sb[h][:, k, c * DS:(c + 1) * DS],
                            start=(k == 0), stop=(k == KT - 1))
                gp = work.tile([P, NF16], F16, name="gp")
                nc.vector.tensor_add(gp[:], ps[:],
                                     b16_sb[:, h * NF16:(h + 1) * NF16])
                nc.scalar.activation(gp[:, 0:DS], gp[:, 0:DS], AF.Tanh)
                if promote_o:
                    nc.scalar.activation(gp[:, DS:2 * DS], gp[:, DS:2 * DS],
                                         AF.Sigmoid)
                    i_t, f_t = ifo[:, 0:DS], ifo[:, DS:2 * DS]
                    g_t, o_t = gp[:, 0:DS], gp[:, DS:2 * DS]
                else:
                    i_t, f_t, o_t = (ifo[:, 0:DS], ifo[:, DS:2 * DS],
                                     ifo[:, 2 * DS:3 * DS])
                    g_t = gp[:, 0:DS]
                ig = work.tile([P, DS], F16, name="ig")
                nc.vector.tensor_mul(ig[:], i_t, g_t)
                cnew = work.tile([P, DS], F16, name="cnew")
                nc.vector.tensor_mul(cnew[:], f_t, cp[:])
                nc.vector.tensor_add(cnew[:], cnew[:], ig[:])
                tct = work.tile([P, DS], F16, name="tct")
                nc.scalar.activation(tct[:], cnew[:], AF.Tanh)
                hnew = work.tile([P, DS], F16, name="hnew")
                nc.vector.tensor_mul(hnew[:], o_t, tct[:])
                nc.sync.dma_start(
                    c_d[b * P:(b + 1) * P, h * DS:(h + 1) * DS], cnew[:])
                nc.sync.dma_start(
                    h_d[b * P:(b + 1) * P, h * DS:(h + 1) * DS], hnew[:])

            # --- units 0/1 fp8: warmup dummies, then k-outer DMA chase.
            ps01 = [ps8_pool.tile([P, NF8], F32, name="ps8") for _ in range(2)]
            for i in range(8):
                nc.tensor.matmul(ps01[0][:, (i % 2) * DS:(i % 2 + 1) * DS],
                                 scratch[:, 0:P], scratch[:, P:],
                                 start=True, stop=True)
            for kp in range(KP):
                for u in range(2):
                    mm8(ps01[u], 0, kp, u)
            ifos = {0: fp8_epi(0, ps01[0]), 1: fp8_epi(1, ps01[1])}

            # staged lower-priority DMA, attached to unit slots: each entry
            # is (emit_after_fp8_of_unit, list of thunks).
            def stage(u):
                if u == 2:
                    for kp in range(KP):
                        dma_w16(0, kp)
                        dma_x(x16_sb, x16_d, kp)
                elif 4 <= u <= 7:
                    for kp in range(2 * (u - 4), 2 * (u - 4) + 2):
                        dma_w8(1, kp)
                elif u == 8:
                    nc.sync.dma_start(b8_sb[:, NF8:], b8_d[:, NF8:])
                    nc.sync.dma_start(b16_sb[:, NF16:], b16_d[:, NF16:])
                elif 9 <= u <= 12:
                    for kp in range(2 * (u - 9), 2 * (u - 9) + 2):
                        dma_w16(1, kp)

            # --- steady state: fp8(u) dense; g pipe trails by GLAG units.
            for u in range(2, NU + GLAG):
                if u < NU:
                    ps = fp8_chain(u)
                    stage(u)
                    ifos[u] = fp8_epi(u, ps)
                j = u - GLAG
                if j >= 0:
                    g_chain_and_join(j, ifos.pop(j))

    nc.compile()
    return nc


def _e4m3(v):
    return np.clip(v, -240.0, 240.0).astype(ml_dtypes.float8_e4m3fn)


def _pmajor(a_kp, kt=KT, p=P):
    """[KT*P, N] k-major rows -> [P, KT*N] partition-major."""
    n = a_kp.shape[1]
    return np.ascontiguousarray(
        a_kp.reshape(kt, p, n).transpose(1, 0, 2).reshape(p, kt * n))


def prep_inputs(input, h_prev, c_prev, W_ih, b_ih, W_hh, b_hh,
                n_cores=N_CORES, promote_o=PROMOTE_O):
    """Host-side shard + layout + quantization prep (not in HW exec time)."""
    NF8, NF16 = _cfg(promote_o)
    input = np.asarray(input, np.float32)
    h_prev = np.asarray(h_prev, np.float32)
    c_prev16 = np.asarray(c_prev, np.float16)
    W_cat = np.concatenate([np.asarray(W_ih, np.float32),
                            np.asarray(W_hh, np.float32)], axis=1)  # [G, K]
    bias = (np.asarray(b_ih, np.float32) + np.asarray(b_hh, np.float32))

    H = H_DIM
    blocks8 = [0, 1] if promote_o else [0, 1, 3]   # gate row-blocks i,f,(o)
    blocks16 = [2, 3] if promote_o else [2]        # g,(o)
    idx8, idx16 = [], []
    for hh in range(NH):
        for gb in blocks8:
            idx8 += list(range(gb * H + hh * DS, gb * H + (hh + 1) * DS))
        for gb in blocks16:
            idx16 += list(range(gb * H + hh * DS, gb * H + (hh + 1) * DS))

    # [K, cols] k-major, with the half-h blocks interleaved per k-tile in the
    # partition-major transform: cols order is [h][kt] major on the DRAM side.
    w8_k = _e4m3(W_cat[idx8, :].T * SW)            # [K, NH*NF8]
    w16_k = W_cat[idx16, :].T.astype(np.float16)
    # rearrange to [P, NH*KT*NF] with [h][kt][col] ordering
    w8 = np.concatenate(
        [_pmajor(np.ascontiguousarray(w8_k[:, h * NF8:(h + 1) * NF8]))
         for h in range(NH)], axis=1)
    w16 = np.concatenate(
        [_pmajor(np.ascontiguousarray(w16_k[:, h * NF16:(h + 1) * NF16]))
         for h in range(NH)], axis=1)

    bias8 = np.ascontiguousarray(np.broadcast_to(
        (bias[idx8] * SCALE).astype(np.float16), (P, NH * NF8)))
    bias16 = np.ascontiguousarray(np.broadcast_to(
        bias[idx16].astype(np.float16), (P, NH * NF16)))

    xh = np.concatenate([input, h_prev], axis=1)    # [B, K]
    xhT = xh.T                                      # [K, B] view

    b_loc = input.shape[0] // n_cores
    in_maps = []
    for c in range(n_cores):
        sl = np.ascontiguousarray(xhT[:, c * b_loc:(c + 1) * b_loc])
        in_maps.append({
            "x8": _pmajor(_e4m3(sl * SX)),
            "x16": _pmajor(sl.astype(np.float16)),
            "w8": w8,
            "w16": w16,
            "bias8": bias8,
            "bias16": bias16,
            "c_prev": np.ascontiguousarray(
                c_prev16[c * b_loc:(c + 1) * b_loc]),
        })
    return in_maps


def run_lstm(inputs, trace=False, **spmd_kwargs):
    """Builds + runs the kernel on all 8 cores. Returns (h_t, c_t), results."""
    in_maps = prep_inputs(**inputs)
    nc = build_lstm_nc()
    res = run_bass_kernel_spmd(nc, in_maps, core_ids=list(range(N_CORES)),
                               trace=trace, **spmd_kwargs)
    h_t = np.concatenate([r["h_out"] for r in res.results],
                         axis=0).astype(np.float32)
    c_t = np.concatenate([r["c_out"] for r in res.results],
                         axis=0).astype(np.float32)
    return (h_t, c_t), res


def kernel(input, h_prev, c_prev, W_ih, b_ih, W_hh, b_hh):
    (h_t, c_t), _ = run_lstm(dict(
        input=input, h_prev=h_prev, c_prev=c_prev,
        W_ih=W_ih, b_ih=b_ih, W_hh=W_hh, b_hh=b_hh))
    return (h_t, c_t)
